# revision 20
# baseline (speedup 1.0000x reference)
"""Bass/Tile TRN2 kernel for nn_Attn: out = softmax_s(hidden . (W @ enc + b)).

Math: energies[b,s] = hidden[b] . (W enc[s,b] + bias) = (hidden[b] W) . enc[s,b] + const(b).
The const(b) term cancels in the softmax (and attn_b is zeros anyway), so per
batch element b:
    v_b = hidden[b] @ W            (tiny [1,H]x[H,H] GEMM)
    E[s] = enc[s, b, :] . v_b      (the 16.8 MB/core stream — the roofline)
    out[b, 0, :] = softmax_s(E)

Sharding: data-parallel over batch (core b owns batch b) for enc; the [H,H]
weight is sharded 8-way by o-rows instead of replicated (4 MiB -> 0.5 MiB per
core): each core computes partial v for ALL 8 batches from its o-slab on PE,
then one ReduceScatter ([8,H] -> [1,H]) hands core b exactly its own summed
v_b. The collective runs on the (modeled) collective cores, overlapped with
the enc stream, so the per-core DMA bill drops from 21 MiB to 17.3 MiB.

Energies on PE (not DVE): host pre-transposes enc to encT[h, s'] with
s' = t*64 + p <-> s = p*64 + t, streamed as 8 h-chunk buffers [128, 4096].
E[:, t] ([64, 64] layout) accumulates in PSUM over the 8 h-chunks via
[128K x 64M x 1N] matmuls (512 total), trailing the DMA stream by well under
a microsecond. Softmax: shift/exp/sums for the first 7 s-blocks' columns run
while the last enc block is in flight; only the last block's exps, the
normalize, and the output write trail the last byte. The cross-partition sum
uses a ones-matmul on PE (sum+broadcast in one op, cheaper than gpsimd on
the tail); the output leaves via a dma_scatter_add whose descriptors are
pre-generated at kernel start (scatter-add onto the pre-zeroed output region
== plain write), so only a trigger_dma (no HWDGE/DGE latency) sits in the
tail. The [64, 64] E layout exists because scatter rows must be 256-byte
aligned.

Host-side layout trickery (shard_inputs): W's columns are permuted so the
ReduceScatter output lands PE-ready: colperm[j*128 + n] = (n%8)*128 + j*16 +
n//8 makes v arrive as v_lin[p*8 + j] = v_b[j*128 + p], i.e. a plain
[128, 8] row-major load whose column j is exactly the h-chunk-j operand.

Post-compile fixups (_fix_scatter_sem): the prepare_only scatter's
descriptor-completion sem is rewritten to the DMASW lane sem tile assigned
(tile's epilogue waits that lane, but the prepare_only API can only bake a
user sem), and the epilogue's scatter wait is rotated to the end of the SP
wait run so the cheap lane waits decode while the scatter is in flight.
"""

import numpy as np

import concourse.bass as bass
import concourse.mybir as mybir
import concourse.tile as tile
from concourse import bacc
from concourse.bass_isa import ReduceOp
from concourse.bass_utils import run_bass_kernel_spmd

S, B, H = 4096, 8, 1024
P = 128
NCORES = 8
EP = 64               # E-tile partitions (s = p*64 + t, p in [0,64))
SCH = S // EP         # 64 energy columns per partition
NCH = H // P          # 8 h-chunks
SB = 512              # s-columns per enc DMA ([128, 512] = 256 KiB, 728 ns)
NSB = S // SB         # 8 s-blocks
TPB = SB // EP        # 8 s-tiles (PE M-tiles) per s-block

_cached_nc = None


def _build():
    nc = bacc.Bacc(
        "TRN2", target_bir_lowering=False, debug=False, num_devices=NCORES
    )
    f32 = mybir.dt.float32
    encT_d = nc.dram_tensor("encT", [H, S], f32, kind="ExternalInput")
    w_d = nc.dram_tensor("wslab", [P, H + B + 2], f32, kind="ExternalInput")
    out_d = nc.dram_tensor("out", [S], f32, kind="ExternalOutput")
    cc_in = nc.dram_tensor("cc_in", [B, H], f32, kind="Internal")
    cc_out = nc.dram_tensor("cc_out", [1, H], f32, kind="Internal")

    encT_r = encT_d.ap().rearrange("(c k) s -> c k s", c=NCH)  # [8, 128, 4096]
    out_zr = out_d.ap().rearrange("(p q) -> p q", p=32)        # [32, 128] zero-fill view (512B rows)
    out_sc = out_d.ap().rearrange("(r e) -> r e", e=EP)        # [64, 64] scatter rows
    vh_src = cc_out.ap().rearrange("one (p j) -> (one p) j", p=P)  # [128, 8]

    with tile.TileContext(nc) as tc:
        with (
            tc.tile_pool(name="enc", bufs=1) as encp,
            tc.tile_pool(name="small", bufs=1) as small,
            tc.tile_pool(name="psum", bufs=1, space=bass.MemorySpace.PSUM) as psum,
        ):
            # ---- v pipeline: partial vT for all 8 batches from this core's
            # o-slab, ReduceScatter, then a PE-ready [128, 8] reload.
            # one packed load: cols 0..1023 = W o-slab, 1024..1031 = hidT
            # (f32), 1032..1033 = scatter idxs (int16 x4, bitcast view)
            wpk = small.tile([P, H + B + 2], f32)
            nc.sync.dma_start(wpk[:], w_d.ap())
            wslab = wpk
            pk_off = H
            # ---- out path, prepared off the critical tail: pre-generate
            # scatter-add descriptors (scatter-add onto the zero-filled
            # output == plain write), fire via trigger_dma after the
            # normalize; tile's count=None path wires the data deps.
            zt = small.tile([32, P], f32)
            nc.vector.memset(zt[:], 0.0)
            outt = small.tile([P, SCH], f32)
            nc.vector.memset(outt[:], 0.0)
            dsem = nc.alloc_semaphore("dsem")
            nc.gpsimd.dma_scatter_add(
                out_sc,
                outt[:].rearrange("p (a f) -> p a f", a=1),
                wpk[:, pk_off + B : pk_off + B + 2].bitcast(mybir.dt.int16),
                num_idxs=EP,
                num_idxs_reg=EP,
                elem_size=EP,
                prepare_only=True,
                sem=dsem,
            )

            vps = psum.tile([B, H], f32)
            for j in range(NCH):
                nc.tensor.matmul(
                    vps[:, j * P : (j + 1) * P],
                    wpk[:, pk_off : pk_off + B],
                    wslab[:, j * P : (j + 1) * P],
                    start=True,
                    stop=True,
                )
            vsb = small.tile([B, H], f32)
            nc.vector.tensor_copy(vsb[:], vps[:])
            nc.gpsimd.dma_start(cc_in.ap(), vsb[:])
            nc.gpsimd.collective_compute(
                "ReduceScatter",
                mybir.AluOpType.add,
                [list(range(NCORES))],
                [cc_in.ap()],
                [cc_out.ap()],
            )
            vh = small.tile([P, B], f32)  # vh[p, j] = v_b[j*128 + p]
            nc.gpsimd.dma_start(vh[:], vh_src)


            # ---- enc stream: 8 resident h-chunk buffers, blocks of 512 s,
            # alternating two HWDGE queues
            qs = [nc.sync, nc.scalar]
            qi = 0
            encb = encp.tile([P, NCH, S], f32)
            for bs in range(NSB):
                sl = slice(bs * SB, (bs + 1) * SB)
                for c in range(NCH):
                    qs[qi % 2].dma_start(encb[:, c, sl], encT_r[c, :, sl])
                    qi += 1

            # zero-fill rides after the enc stream: its only consumer is the
            # scatter trigger at the very end, while the last enc byte gates
            # the energy/softmax tail.
            nc.sync.dma_start(out_zr, zt[:])

            # ---- energies: E[:, t] = sum_c encT_c[:, t-tile]^T @ vh[:, c].
            # The last s-block's 8 columns get their own PSUM tile so the
            # softmax prep's deps stop at the block boundary (col 55).
            NTL = TPB  # tail columns = one s-block
            E_ps = psum.tile([EP, SCH - NTL], f32)
            Etl = psum.tile([EP, NTL], f32)
            for t in range(SCH):
                dst = (
                    E_ps[:, t : t + 1]
                    if t < SCH - NTL
                    else Etl[:, t - (SCH - NTL) : t - (SCH - NTL) + 1]
                )
                for c in range(NCH):
                    nc.tensor.matmul(
                        dst,
                        encb[:, c, t * EP : (t + 1) * EP],
                        vh[:, c : c + 1],
                        start=(c == 0),
                        stop=(c == NCH - 1),
                    )

            # ---- softmax: shift/exp/sums for cols 0..27 run while the last
            # enc block is in flight; only the last block's 4 exps + the
            # normalize trail the last byte. The shift m~ = global max of
            # cols 0..27 is exact for softmax (shift-invariance); exp(E - m~)
            # of the last 4 cols cannot overflow for randn energies (would
            # need a > 88 logit gap). Cross-partition sum+broadcast via a
            # ones-matmul on PE (cheaper than gpsimd on the critical tail).
            m1 = small.tile([EP, 1], f32)
            negm = small.tile([EP, 1], f32)
            expt = small.tile([EP, SCH], f32)
            sums = small.tile([EP, 1], f32)
            stl = small.tile([EP, 1], f32)
            ones = small.tile([EP, EP], f32)
            nc.vector.memset(ones[:], 1.0)
            nc.vector.reduce_max(m1[:], E_ps[:], axis=mybir.AxisListType.X)
            nc.gpsimd.partition_all_reduce(m1[:], m1[:], EP, ReduceOp.max)
            nc.scalar.mul(negm[:], m1[:], -1.0)
            nc.scalar.activation(
                expt[:, 0 : SCH - NTL],
                E_ps[:],
                mybir.ActivationFunctionType.Exp,
                bias=negm[:],
                accum_out=sums[:],
            )
            nc.scalar.activation(
                expt[:, SCH - NTL : SCH],
                Etl[:],
                mybir.ActivationFunctionType.Exp,
                bias=negm[:],
                accum_out=stl[:],
            )
            nc.vector.tensor_add(sums[:], sums[:], stl[:])
            total_ps = psum.tile([EP, 1], f32)
            nc.tensor.matmul(total_ps[:], ones[:], sums[:], start=True, stop=True)
            rs = small.tile([EP, 1], f32)
            nc.vector.reciprocal(rs[:], total_ps[:])
            nc.vector.tensor_scalar_mul(outt[0:EP, :], expt[:], rs[:])
            nc.gpsimd.trigger_dma(None)

    nc.compile()
    _fix_scatter_sem(nc)
    return nc


def _fix_scatter_sem(nc):
    """Point the scatter prep's descriptor-completion sem (on_update[0], our
    placeholder dsem) at the DMASW lane sem tile assigned to the prep. Tile's
    epilogue waits on that lane sem, but only the descriptor-baked sem fires
    at DMA completion — they must be the same sem, which the prepare_only API
    can't express (the lane is assigned during lowering)."""
    fn = nc.m.functions[0]
    insts = [i for bb in fn.blocks for i in bb.instructions]
    waited = {}
    updated = set()
    prep = None
    for i in insts:
        si = i.sync_info
        if not si:
            continue
        for u in si.on_update or []:
            updated.add(u.id)
        for w in si.on_wait or []:
            waited.setdefault(w.id, []).append(i.name)
        if type(i).__name__ == "InstDMAScatterAddAnt":
            prep = i
    assert prep is not None
    orphans = [sid for sid in waited if sid not in updated]
    assert len(orphans) == 1, (orphans, {k: waited[k] for k in orphans})
    si = prep.sync_info
    upd = list(si.on_update)
    first = upd[0]
    upd[0] = first.__replace__(id=orphans[0])
    prep.sync_info = si.__replace__(on_update=upd)


    # The scatter completes last (its trigger fires after the normalize), but
    # tile emitted its epilogue wait FIRST in the SP wait run — the ~8
    # trailing 50ns wait decodes then serialize after it. Rotate it to the
    # end of its run so the cheap waits decode while the scatter is in
    # flight.
    lane_id = orphans[0]
    for bb in fn.blocks:
        bl = list(bb.instructions)
        io = None
        for k, i in enumerate(bl):
            si2 = i.sync_info
            if (
                type(i).__name__ == "InstEventSemaphore"
                and si2
                and any(w.id == lane_id for w in (si2.on_wait or []))
            ):
                io = k
                break
        if io is None:
            continue
        ie = io + 1
        while (
            ie < len(bl)
            and type(bl[ie]).__name__ == "InstEventSemaphore"
            and bl[ie].engine == bl[io].engine
        ):
            ie += 1
        if ie > io + 1:
            inst = bl.pop(io)
            bl.insert(ie - 1, inst)
            bb.instructions = bl
        break

    # The framework's const-AP memsets (no readers in this kernel) sit ahead
    # of the entry barrier and delay its release by ~0.4us. Move them past
    # the barrier so the first DMA issues immediately.
    bl0 = list(fn.blocks[0].instructions)
    movers = [
        i for i in bl0
        if type(i).__name__ == "InstMemset"
        and i.outs
        and getattr(
            getattr(getattr(i.outs[0], "bass_ap", None), "tensor", None),
            "name",
            "",
        ).startswith("const-")
    ]
    if movers:
        keep = [i for i in bl0 if i not in movers]
        fn.blocks[0].instructions = keep
        bl1 = list(fn.blocks[1].instructions)
        fn.blocks[1].instructions = movers + bl1

    # With the memsets gone, block 0's entry barrier synchronizes
    # nothing-to-nothing (exit barriers reuse the same gather/release sems
    # and the protocol is self-cleaning: both sems return to 0), so drop it
    # — engines branch straight into the body.
    bl0 = list(fn.blocks[0].instructions)
    fn.blocks[0].instructions = [
        i for i in bl0
        if type(i).__name__ not in ("InstDrain", "InstEventSemaphore")
    ]

    # The exit emits barrier / sem-reset / barrier. Round 2 only holds the
    # engines until Pool's reset retires, which the NEFF-completion protocol
    # (all queues drained before the next invocation) already guarantees —
    # drop it. Round 1 stays: it orders the reset after every in-flight sem
    # wait.
    lastbb = fn.blocks[-1]
    bl = list(lastbb.instructions)
    reset_idx = None
    for k, i in enumerate(bl):
        if type(i).__name__ == "InstDrain" and getattr(i, "is_reset_sema", False):
            reset_idx = k
    assert reset_idx is not None
    def _is_barrier(i):
        if type(i).__name__ not in ("InstDrain", "InstEventSemaphore"):
            return False
        si2 = i.sync_info
        if not si2:
            return False
        names = [w.ant_name or "" for w in (si2.on_wait or [])] + [
            u.ant_name or "" for u in (si2.on_update or [])
        ]
        return any(n.startswith("barrier_") for n in names)
    lastbb.instructions = bl[: reset_idx + 1] + [
        i for i in bl[reset_idx + 1 :] if not _is_barrier(i)
    ]


def _get_nc():
    global _cached_nc
    if _cached_nc is None:
        _cached_nc = _build()
    return _cached_nc


def _colperm():
    """colperm[j*128 + n] = (n%8)*128 + j*16 + n//8 — makes the RS output
    arrive p-major (v_lin[p*8 + j] = v[j*128 + p])."""
    hp = np.arange(H)
    j, n = hp // P, hp % P
    return (n % NCH) * P + j * (P // NCH) + n // NCH


def shard_inputs(inputs):
    """Per-core maps: core b gets batch b's enc (transposed + s-permuted to
    the PE tile layout), its o-slab of W (columns permuted for the RS->PE
    handoff), and the full hidden pre-transposed to the matmul lhsT layout."""
    hidden = np.asarray(inputs["hidden"], dtype=np.float32)
    enc = np.asarray(inputs["encoder_outputs"], dtype=np.float32)
    w = np.asarray(inputs["attn_w"], dtype=np.float32)
    # attn_b is a constant shift across s per batch -> cancels in softmax.
    w_perm = w[:, _colperm()]
    in_maps = []
    sa = np.zeros((P, 4), dtype=np.int16)
    for i in range(EP):
        sa[i % 16, i // 16] = i
    for b in range(NCORES):
        # encT[h, t*64 + p] = enc[p*64 + t, b, h]
        encT = np.ascontiguousarray(
            enc[:, b, :].reshape(EP, SCH, H).transpose(2, 1, 0).reshape(H, S)
        )
        in_maps.append(
            {
                "encT": encT,
                "wslab": np.ascontiguousarray(
                    np.concatenate(
                        [
                            w_perm[b * P : (b + 1) * P, :],
                            hidden[0, :, b * P : (b + 1) * P].T.astype(np.float32),
                            sa.view(np.float32),
                        ],
                        axis=1,
                    )
                ),
            }
        )
    return in_maps


def run(inputs, trace=False):
    """Shard, run SPMD on 8 cores, gather. Returns (output, BassKernelResults)."""
    nc = _get_nc()
    in_maps = shard_inputs(inputs)
    res = run_bass_kernel_spmd(
        nc, in_maps, core_ids=list(range(NCORES)), trace=trace
    )
    out = np.stack([res.results[b]["out"] for b in range(NCORES)], axis=0)
    return out[:, None, :].astype(np.float32), res


def kernel(hidden, encoder_outputs, attn_w, attn_b=None, **_unused):
    out, _ = run(
        {
            "hidden": hidden,
            "encoder_outputs": encoder_outputs,
            "attn_w": attn_w,
        }
    )
    return out


# revision 21
# speedup vs baseline: 1.0020x; 1.0020x over previous
"""Bass/Tile TRN2 kernel for nn_Attn: out = softmax_s(hidden . (W @ enc + b)).

Math: energies[b,s] = hidden[b] . (W enc[s,b] + bias) = (hidden[b] W) . enc[s,b] + const(b).
The const(b) term cancels in the softmax (and attn_b is zeros anyway), so per
batch element b:
    v_b = hidden[b] @ W            (tiny [1,H]x[H,H] GEMM)
    E[s] = enc[s, b, :] . v_b      (the 16.8 MB/core stream — the roofline)
    out[b, 0, :] = softmax_s(E)

Sharding: data-parallel over batch (core b owns batch b) for enc; the [H,H]
weight is sharded 8-way by o-rows instead of replicated (4 MiB -> 0.5 MiB per
core): each core computes partial v for ALL 8 batches from its o-slab on PE,
then one ReduceScatter ([8,H] -> [1,H]) hands core b exactly its own summed
v_b. The collective runs on the (modeled) collective cores, overlapped with
the enc stream, so the per-core DMA bill drops from 21 MiB to 17.3 MiB.

Energies on PE (not DVE): host pre-transposes enc to encT[h, s'] with
s' = t*64 + p <-> s = p*64 + t, streamed as 8 h-chunk buffers [128, 4096].
E[:, t] ([64, 64] layout) accumulates in PSUM over the 8 h-chunks via
[128K x 64M x 1N] matmuls (512 total), trailing the DMA stream by well under
a microsecond. Softmax: shift/exp/sums for the first 7 s-blocks' columns run
while the last enc block is in flight; only the last block's exps, the
normalize, and the output write trail the last byte. The cross-partition sum
uses a ones-matmul on PE (sum+broadcast in one op, cheaper than gpsimd on
the tail); the output leaves via a dma_scatter_add whose descriptors are
pre-generated at kernel start (scatter-add onto the pre-zeroed output region
== plain write), so only a trigger_dma (no HWDGE/DGE latency) sits in the
tail. The [64, 64] E layout exists because scatter rows must be 256-byte
aligned.

Host-side layout trickery (shard_inputs): W's columns are permuted so the
ReduceScatter output lands PE-ready: colperm[j*128 + n] = (n%8)*128 + j*16 +
n//8 makes v arrive as v_lin[p*8 + j] = v_b[j*128 + p], i.e. a plain
[128, 8] row-major load whose column j is exactly the h-chunk-j operand.

Post-compile fixups (_fix_scatter_sem): the prepare_only scatter's
descriptor-completion sem is rewritten to the DMASW lane sem tile assigned
(tile's epilogue waits that lane, but the prepare_only API can only bake a
user sem), and the epilogue's scatter wait is rotated to the end of the SP
wait run so the cheap lane waits decode while the scatter is in flight.
"""

import numpy as np

import concourse.bass as bass
import concourse.mybir as mybir
import concourse.tile as tile
from concourse import bacc
from concourse.bass_isa import ReduceOp
from concourse.bass_utils import run_bass_kernel_spmd

S, B, H = 4096, 8, 1024
P = 128
NCORES = 8
EP = 64               # E-tile partitions (s = p*64 + t, p in [0,64))
SCH = S // EP         # 64 energy columns per partition
NCH = H // P          # 8 h-chunks
SB = 512              # s-columns per enc DMA ([128, 512] = 256 KiB, 728 ns)
NSB = S // SB         # 8 s-blocks
TPB = SB // EP        # 8 s-tiles (PE M-tiles) per s-block

_cached_nc = None


def _build():
    nc = bacc.Bacc(
        "TRN2", target_bir_lowering=False, debug=False, num_devices=NCORES
    )
    f32 = mybir.dt.float32
    encT_d = nc.dram_tensor("encT", [H, S], f32, kind="ExternalInput")
    w_d = nc.dram_tensor("wslab", [P, H + B + 2], f32, kind="ExternalInput")
    out_d = nc.dram_tensor("out", [S], f32, kind="ExternalOutput")
    cc_in = nc.dram_tensor("cc_in", [B, H], f32, kind="Internal")
    cc_out = nc.dram_tensor("cc_out", [1, H], f32, kind="Internal")

    encT_r = encT_d.ap().rearrange("(c k) s -> c k s", c=NCH)  # [8, 128, 4096]
    out_zr = out_d.ap().rearrange("(p q) -> p q", p=32)        # [32, 128] zero-fill view (512B rows)
    out_sc = out_d.ap().rearrange("(r e) -> r e", e=EP)        # [64, 64] scatter rows
    vh_src = cc_out.ap().rearrange("one (p j) -> (one p) j", p=P)  # [128, 8]

    with tile.TileContext(nc) as tc:
        with (
            tc.tile_pool(name="enc", bufs=1) as encp,
            tc.tile_pool(name="small", bufs=1) as small,
            tc.tile_pool(name="psum", bufs=1, space=bass.MemorySpace.PSUM) as psum,
        ):
            # ---- v pipeline: partial vT for all 8 batches from this core's
            # o-slab, ReduceScatter, then a PE-ready [128, 8] reload.
            # one packed load: cols 0..1023 = W o-slab, 1024..1031 = hidT
            # (f32), 1032..1033 = scatter idxs (int16 x4, bitcast view)
            wpk = small.tile([P, H + B + 2], f32)
            nc.sync.dma_start(wpk[:], w_d.ap())
            wslab = wpk
            pk_off = H
            # ---- out path, prepared off the critical tail: pre-generate
            # scatter-add descriptors (scatter-add onto the zero-filled
            # output == plain write), fire via trigger_dma after the
            # normalize; tile's count=None path wires the data deps.
            zt = small.tile([32, P], f32)
            nc.vector.memset(zt[:], 0.0)
            outt = small.tile([P, SCH], f32)
            nc.vector.memset(outt[:], 0.0)
            dsem = nc.alloc_semaphore("dsem")
            nc.gpsimd.dma_scatter_add(
                out_sc,
                outt[:].rearrange("p (a f) -> p a f", a=1),
                wpk[:, pk_off + B : pk_off + B + 2].bitcast(mybir.dt.int16),
                num_idxs=EP,
                num_idxs_reg=EP,
                elem_size=EP,
                prepare_only=True,
                sem=dsem,
            )

            vps = psum.tile([B, H], f32)
            for j in range(NCH):
                nc.tensor.matmul(
                    vps[:, j * P : (j + 1) * P],
                    wpk[:, pk_off : pk_off + B],
                    wslab[:, j * P : (j + 1) * P],
                    start=True,
                    stop=True,
                )
            vsb = small.tile([B, H], f32)
            nc.vector.tensor_copy(vsb[:], vps[:])
            nc.gpsimd.dma_start(cc_in.ap(), vsb[:])
            nc.gpsimd.collective_compute(
                "ReduceScatter",
                mybir.AluOpType.add,
                [list(range(NCORES))],
                [cc_in.ap()],
                [cc_out.ap()],
            )
            vh = small.tile([P, B], f32)  # vh[p, j] = v_b[j*128 + p]
            nc.gpsimd.dma_start(vh[:], vh_src)


            # ---- enc stream: 8 resident h-chunk buffers, blocks of 512 s,
            # alternating two HWDGE queues
            qs = [nc.sync, nc.scalar]
            qi = 0
            encb = encp.tile([P, NCH, S], f32)
            for bs in range(NSB):
                sl = slice(bs * SB, (bs + 1) * SB)
                for c in range(NCH):
                    qs[qi % 2].dma_start(encb[:, c, sl], encT_r[c, :, sl])
                    qi += 1

            # zero-fill rides after the enc stream: its only consumer is the
            # scatter trigger at the very end, while the last enc byte gates
            # the energy/softmax tail.
            nc.sync.dma_start(out_zr, zt[:])

            # ---- energies: E[:, t] = sum_c encT_c[:, t-tile]^T @ vh[:, c].
            # The last s-block's 8 columns get their own PSUM tile so the
            # softmax prep's deps stop at the block boundary (col 55).
            NTL = TPB  # tail columns = one s-block
            E_ps = psum.tile([EP, SCH - NTL], f32)
            Etl = psum.tile([EP, NTL], f32)
            for t in range(SCH):
                dst = (
                    E_ps[:, t : t + 1]
                    if t < SCH - NTL
                    else Etl[:, t - (SCH - NTL) : t - (SCH - NTL) + 1]
                )
                for c in range(NCH):
                    nc.tensor.matmul(
                        dst,
                        encb[:, c, t * EP : (t + 1) * EP],
                        vh[:, c : c + 1],
                        start=(c == 0),
                        stop=(c == NCH - 1),
                    )

            # ---- softmax: shift/exp/sums for cols 0..27 run while the last
            # enc block is in flight; only the last block's 4 exps + the
            # normalize trail the last byte. The shift m~ = global max of
            # cols 0..27 is exact for softmax (shift-invariance); exp(E - m~)
            # of the last 4 cols cannot overflow for randn energies (would
            # need a > 88 logit gap). Cross-partition sum+broadcast via a
            # ones-matmul on PE (cheaper than gpsimd on the critical tail).
            m1 = small.tile([EP, 1], f32)
            negm = small.tile([EP, 1], f32)
            expt = small.tile([EP, SCH], f32)
            sums = small.tile([EP, 1], f32)
            stl = small.tile([EP, 1], f32)
            ones = small.tile([EP, EP], f32)
            nc.vector.memset(ones[:], 1.0)
            nc.vector.reduce_max(m1[:], E_ps[:], axis=mybir.AxisListType.X)
            nc.gpsimd.partition_all_reduce(m1[:], m1[:], EP, ReduceOp.max)
            nc.scalar.mul(negm[:], m1[:], -1.0)
            nc.scalar.activation(
                expt[:, 0 : SCH - NTL],
                E_ps[:],
                mybir.ActivationFunctionType.Exp,
                bias=negm[:],
                accum_out=sums[:],
            )
            nc.scalar.activation(
                expt[:, SCH - NTL : SCH],
                Etl[:],
                mybir.ActivationFunctionType.Exp,
                bias=negm[:],
                accum_out=stl[:],
            )
            nc.vector.tensor_add(sums[:], sums[:], stl[:])
            total_ps = psum.tile([EP, 1], f32)
            nc.tensor.matmul(total_ps[:], ones[:], sums[:], start=True, stop=True)
            rs = small.tile([EP, 1], f32)
            nc.vector.reciprocal(rs[:], total_ps[:])
            nc.vector.tensor_scalar_mul(outt[0:EP, :], expt[:], rs[:])
            nc.gpsimd.trigger_dma(None)

    nc.compile()
    _fix_scatter_sem(nc)
    return nc


def _fix_scatter_sem(nc):
    """Point the scatter prep's descriptor-completion sem (on_update[0], our
    placeholder dsem) at the DMASW lane sem tile assigned to the prep. Tile's
    epilogue waits on that lane sem, but only the descriptor-baked sem fires
    at DMA completion — they must be the same sem, which the prepare_only API
    can't express (the lane is assigned during lowering)."""
    fn = nc.m.functions[0]
    insts = [i for bb in fn.blocks for i in bb.instructions]
    waited = {}
    updated = set()
    prep = None
    for i in insts:
        si = i.sync_info
        if not si:
            continue
        for u in si.on_update or []:
            updated.add(u.id)
        for w in si.on_wait or []:
            waited.setdefault(w.id, []).append(i.name)
        if type(i).__name__ == "InstDMAScatterAddAnt":
            prep = i
    assert prep is not None
    orphans = [sid for sid in waited if sid not in updated]
    assert len(orphans) == 1, (orphans, {k: waited[k] for k in orphans})
    si = prep.sync_info
    upd = list(si.on_update)
    first = upd[0]
    upd[0] = first.__replace__(id=orphans[0])
    prep.sync_info = si.__replace__(on_update=upd)


    # The scatter completes last (its trigger fires after the normalize), but
    # tile emitted its epilogue wait FIRST in the SP wait run — the ~8
    # trailing 50ns wait decodes then serialize after it. Rotate it to the
    # end of its run so the cheap waits decode while the scatter is in
    # flight.
    lane_id = orphans[0]
    for bb in fn.blocks:
        bl = list(bb.instructions)
        io = None
        for k, i in enumerate(bl):
            si2 = i.sync_info
            if (
                type(i).__name__ == "InstEventSemaphore"
                and si2
                and any(w.id == lane_id for w in (si2.on_wait or []))
            ):
                io = k
                break
        if io is None:
            continue
        i0 = io
        while (
            i0 > 0
            and type(bl[i0 - 1]).__name__ == "InstEventSemaphore"
            and bl[i0 - 1].engine == bl[io].engine
        ):
            i0 -= 1
        ie = io + 1
        while (
            ie < len(bl)
            and type(bl[ie]).__name__ == "InstEventSemaphore"
            and bl[ie].engine == bl[io].engine
        ):
            ie += 1
        run = bl[i0:ie]
        # Re-pack the exit wait conditions: the two LATE conds (the scatter
        # lane, which fires at transfer+0.9, and Pool_sequencer, which fires
        # at trigger+0.9) go together in the LAST EventSemaphore; all early
        # conds decode serially before the park instead of after the wake.
        conds = [w for i in run for w in (i.sync_info.on_wait or [])]
        late = [
            w
            for w in conds
            if w.id == lane_id or (w.ant_name or "").startswith("Pool_sequencer")
        ]
        early = [w for w in conds if w not in late]
        assert 1 <= len(late) <= 2 and len(late) + len(early) == len(conds)
        packs = []
        for k in range(len(run) - 1):
            take, early = early[:2], early[2:]
            packs.append(take)
        packs.append(early + late)
        assert all(1 <= len(p) <= 2 for p in packs), [len(p) for p in packs]
        for i, p in zip(run, packs):
            i.sync_info = i.sync_info.__replace__(on_wait=p)
        bb.instructions = bl
        break

    # The framework's const-AP memsets (no readers in this kernel) sit ahead
    # of the entry barrier and delay its release by ~0.4us. Move them past
    # the barrier so the first DMA issues immediately.
    bl0 = list(fn.blocks[0].instructions)
    movers = [
        i for i in bl0
        if type(i).__name__ == "InstMemset"
        and i.outs
        and getattr(
            getattr(getattr(i.outs[0], "bass_ap", None), "tensor", None),
            "name",
            "",
        ).startswith("const-")
    ]
    if movers:
        keep = [i for i in bl0 if i not in movers]
        fn.blocks[0].instructions = keep
        bl1 = list(fn.blocks[1].instructions)
        fn.blocks[1].instructions = movers + bl1

    # With the memsets gone, block 0's entry barrier synchronizes
    # nothing-to-nothing (exit barriers reuse the same gather/release sems
    # and the protocol is self-cleaning: both sems return to 0), so drop it
    # — engines branch straight into the body.
    bl0 = list(fn.blocks[0].instructions)
    fn.blocks[0].instructions = [
        i for i in bl0
        if type(i).__name__ not in ("InstDrain", "InstEventSemaphore")
    ]

    # The exit emits barrier / sem-reset / barrier. Round 2 only holds the
    # engines until Pool's reset retires, which the NEFF-completion protocol
    # (all queues drained before the next invocation) already guarantees —
    # drop it. Round 1 stays: it orders the reset after every in-flight sem
    # wait.
    lastbb = fn.blocks[-1]
    bl = list(lastbb.instructions)
    reset_idx = None
    for k, i in enumerate(bl):
        if type(i).__name__ == "InstDrain" and getattr(i, "is_reset_sema", False):
            reset_idx = k
    assert reset_idx is not None
    def _is_barrier(i):
        if type(i).__name__ not in ("InstDrain", "InstEventSemaphore"):
            return False
        si2 = i.sync_info
        if not si2:
            return False
        names = [w.ant_name or "" for w in (si2.on_wait or [])] + [
            u.ant_name or "" for u in (si2.on_update or [])
        ]
        return any(n.startswith("barrier_") for n in names)
    lastbb.instructions = bl[: reset_idx + 1] + [
        i for i in bl[reset_idx + 1 :] if not _is_barrier(i)
    ]


def _get_nc():
    global _cached_nc
    if _cached_nc is None:
        _cached_nc = _build()
    return _cached_nc


def _colperm():
    """colperm[j*128 + n] = (n%8)*128 + j*16 + n//8 — makes the RS output
    arrive p-major (v_lin[p*8 + j] = v[j*128 + p])."""
    hp = np.arange(H)
    j, n = hp // P, hp % P
    return (n % NCH) * P + j * (P // NCH) + n // NCH


def shard_inputs(inputs):
    """Per-core maps: core b gets batch b's enc (transposed + s-permuted to
    the PE tile layout), its o-slab of W (columns permuted for the RS->PE
    handoff), and the full hidden pre-transposed to the matmul lhsT layout."""
    hidden = np.asarray(inputs["hidden"], dtype=np.float32)
    enc = np.asarray(inputs["encoder_outputs"], dtype=np.float32)
    w = np.asarray(inputs["attn_w"], dtype=np.float32)
    # attn_b is a constant shift across s per batch -> cancels in softmax.
    w_perm = w[:, _colperm()]
    in_maps = []
    sa = np.zeros((P, 4), dtype=np.int16)
    for i in range(EP):
        sa[i % 16, i // 16] = i
    for b in range(NCORES):
        # encT[h, t*64 + p] = enc[p*64 + t, b, h]
        encT = np.ascontiguousarray(
            enc[:, b, :].reshape(EP, SCH, H).transpose(2, 1, 0).reshape(H, S)
        )
        in_maps.append(
            {
                "encT": encT,
                "wslab": np.ascontiguousarray(
                    np.concatenate(
                        [
                            w_perm[b * P : (b + 1) * P, :],
                            hidden[0, :, b * P : (b + 1) * P].T.astype(np.float32),
                            sa.view(np.float32),
                        ],
                        axis=1,
                    )
                ),
            }
        )
    return in_maps


def run(inputs, trace=False):
    """Shard, run SPMD on 8 cores, gather. Returns (output, BassKernelResults)."""
    nc = _get_nc()
    in_maps = shard_inputs(inputs)
    res = run_bass_kernel_spmd(
        nc, in_maps, core_ids=list(range(NCORES)), trace=trace
    )
    out = np.stack([res.results[b]["out"] for b in range(NCORES)], axis=0)
    return out[:, None, :].astype(np.float32), res


def kernel(hidden, encoder_outputs, attn_w, attn_b=None, **_unused):
    out, _ = run(
        {
            "hidden": hidden,
            "encoder_outputs": encoder_outputs,
            "attn_w": attn_w,
        }
    )
    return out


# revision 22
# speedup vs baseline: 1.0025x; 1.0005x over previous
"""Bass/Tile TRN2 kernel for nn_Attn: out = softmax_s(hidden . (W @ enc + b)).

Math: energies[b,s] = hidden[b] . (W enc[s,b] + bias) = (hidden[b] W) . enc[s,b] + const(b).
The const(b) term cancels in the softmax (and attn_b is zeros anyway), so per
batch element b:
    v_b = hidden[b] @ W            (tiny [1,H]x[H,H] GEMM)
    E[s] = enc[s, b, :] . v_b      (the 16.8 MB/core stream — the roofline)
    out[b, 0, :] = softmax_s(E)

Sharding: data-parallel over batch (core b owns batch b) for enc; the [H,H]
weight is sharded 8-way by o-rows instead of replicated (4 MiB -> 0.5 MiB per
core): each core computes partial v for ALL 8 batches from its o-slab on PE,
then one ReduceScatter ([8,H] -> [1,H]) hands core b exactly its own summed
v_b. The collective runs on the (modeled) collective cores, overlapped with
the enc stream, so the per-core DMA bill drops from 21 MiB to 17.3 MiB.

Energies on PE (not DVE): host pre-transposes enc to encT[h, s'] with
s' = t*64 + p <-> s = p*64 + t, streamed as 8 h-chunk buffers [128, 4096].
E[:, t] ([64, 64] layout) accumulates in PSUM over the 8 h-chunks via
[128K x 64M x 1N] matmuls (512 total), trailing the DMA stream by well under
a microsecond. Softmax: shift/exp/sums for the first 7 s-blocks' columns run
while the last enc block is in flight; only the last block's exps, the
normalize, and the output write trail the last byte. The cross-partition sum
uses a ones-matmul on PE (sum+broadcast in one op, cheaper than gpsimd on
the tail); the output leaves via a dma_scatter_add whose descriptors are
pre-generated at kernel start (scatter-add onto the pre-zeroed output region
== plain write), so only a trigger_dma (no HWDGE/DGE latency) sits in the
tail. The [64, 64] E layout exists because scatter rows must be 256-byte
aligned.

Host-side layout trickery (shard_inputs): W's columns are permuted so the
ReduceScatter output lands PE-ready: colperm[j*128 + n] = (n%8)*128 + j*16 +
n//8 makes v arrive as v_lin[p*8 + j] = v_b[j*128 + p], i.e. a plain
[128, 8] row-major load whose column j is exactly the h-chunk-j operand.

Post-compile fixups (_fix_scatter_sem): the prepare_only scatter's
descriptor-completion sem is rewritten to the DMASW lane sem tile assigned
(tile's epilogue waits that lane, but the prepare_only API can only bake a
user sem), and the epilogue's scatter wait is rotated to the end of the SP
wait run so the cheap lane waits decode while the scatter is in flight.
"""

import numpy as np

import concourse.bass as bass
import concourse.mybir as mybir
import concourse.tile as tile
from concourse import bacc
from concourse.bass_isa import ReduceOp
from concourse.bass_utils import run_bass_kernel_spmd

S, B, H = 4096, 8, 1024
P = 128
NCORES = 8
EP = 64               # E-tile partitions (s = p*64 + t, p in [0,64))
SCH = S // EP         # 64 energy columns per partition
NCH = H // P          # 8 h-chunks
SB = 512              # s-columns per enc DMA ([128, 512] = 256 KiB, 728 ns)
NSB = S // SB         # 8 s-blocks
TPB = SB // EP        # 8 s-tiles (PE M-tiles) per s-block

_cached_nc = None


def _build():
    nc = bacc.Bacc(
        "TRN2", target_bir_lowering=False, debug=False, num_devices=NCORES
    )
    f32 = mybir.dt.float32
    encT_d = nc.dram_tensor("encT", [H, S], f32, kind="ExternalInput")
    w_d = nc.dram_tensor("wslab", [P, H + B + 2], f32, kind="ExternalInput")
    out_d = nc.dram_tensor("out", [S], f32, kind="ExternalOutput")
    cc_in = nc.dram_tensor("cc_in", [B, H], f32, kind="Internal")
    cc_out = nc.dram_tensor("cc_out", [1, H], f32, kind="Internal")

    encT_r = encT_d.ap().rearrange("(c k) s -> c k s", c=NCH)  # [8, 128, 4096]
    out_zr = out_d.ap().rearrange("(p q) -> p q", p=32)        # [32, 128] zero-fill view (512B rows)
    out_sc = out_d.ap().rearrange("(r e) -> r e", e=EP)        # [64, 64] scatter rows
    vh_src = cc_out.ap().rearrange("one (p j) -> (one p) j", p=P)  # [128, 8]

    with tile.TileContext(nc) as tc:
        with (
            tc.tile_pool(name="enc", bufs=1) as encp,
            tc.tile_pool(name="small", bufs=1) as small,
            tc.tile_pool(name="psum", bufs=1, space=bass.MemorySpace.PSUM) as psum,
        ):
            # ---- v pipeline: partial vT for all 8 batches from this core's
            # o-slab, ReduceScatter, then a PE-ready [128, 8] reload.
            # one packed load: cols 0..1023 = W o-slab, 1024..1031 = hidT
            # (f32), 1032..1033 = scatter idxs (int16 x4, bitcast view)
            wpk = small.tile([P, H + B + 2], f32)
            nc.sync.dma_start(wpk[:], w_d.ap())
            wslab = wpk
            pk_off = H
            # ---- out path, prepared off the critical tail: pre-generate
            # scatter-add descriptors (scatter-add onto the zero-filled
            # output == plain write), fire via trigger_dma after the
            # normalize; tile's count=None path wires the data deps.
            zt = small.tile([32, P], f32)
            nc.vector.memset(zt[:], 0.0)
            outt = small.tile([P, SCH], f32)
            nc.vector.memset(outt[:], 0.0)
            dsem = nc.alloc_semaphore("dsem")
            nc.gpsimd.dma_scatter_add(
                out_sc,
                outt[:].rearrange("p (a f) -> p a f", a=1),
                wpk[:, pk_off + B : pk_off + B + 2].bitcast(mybir.dt.int16),
                num_idxs=EP,
                num_idxs_reg=EP,
                elem_size=EP,
                prepare_only=True,
                sem=dsem,
            )

            vps = psum.tile([B, H], f32)
            for j in range(NCH):
                nc.tensor.matmul(
                    vps[:, j * P : (j + 1) * P],
                    wpk[:, pk_off : pk_off + B],
                    wslab[:, j * P : (j + 1) * P],
                    start=True,
                    stop=True,
                )
            vsb = small.tile([B, H], f32)
            nc.vector.tensor_copy(vsb[:], vps[:])
            nc.gpsimd.dma_start(cc_in.ap(), vsb[:])
            nc.gpsimd.collective_compute(
                "ReduceScatter",
                mybir.AluOpType.add,
                [list(range(NCORES))],
                [cc_in.ap()],
                [cc_out.ap()],
            )
            vh = small.tile([P, B], f32)  # vh[p, j] = v_b[j*128 + p]
            nc.gpsimd.dma_start(vh[:], vh_src)


            # ---- enc stream: 8 resident h-chunk buffers, blocks of 512 s,
            # alternating two HWDGE queues
            qs = [nc.sync, nc.scalar]
            qi = 0
            encb = encp.tile([P, NCH, S], f32)
            for bs in range(NSB):
                sl = slice(bs * SB, (bs + 1) * SB)
                for c in range(NCH):
                    qs[qi % 2].dma_start(encb[:, c, sl], encT_r[c, :, sl])
                    qi += 1

            # zero-fill rides after the enc stream: its only consumer is the
            # scatter trigger at the very end, while the last enc byte gates
            # the energy/softmax tail.
            nc.sync.dma_start(out_zr, zt[:])

            # ---- energies: E[:, t] = sum_c encT_c[:, t-tile]^T @ vh[:, c].
            # The last s-block's 8 columns get their own PSUM tile so the
            # softmax prep's deps stop at the block boundary (col 55).
            NTL = TPB  # tail columns = one s-block
            E_ps = psum.tile([EP, SCH - NTL], f32)
            Etl = psum.tile([EP, NTL], f32)
            for t in range(SCH):
                dst = (
                    E_ps[:, t : t + 1]
                    if t < SCH - NTL
                    else Etl[:, t - (SCH - NTL) : t - (SCH - NTL) + 1]
                )
                for c in range(NCH):
                    nc.tensor.matmul(
                        dst,
                        encb[:, c, t * EP : (t + 1) * EP],
                        vh[:, c : c + 1],
                        start=(c == 0),
                        stop=(c == NCH - 1),
                    )

            # ---- softmax: shift/exp/sums for cols 0..27 run while the last
            # enc block is in flight; only the last block's 4 exps + the
            # normalize trail the last byte. The shift m~ = global max of
            # cols 0..27 is exact for softmax (shift-invariance); exp(E - m~)
            # of the last 4 cols cannot overflow for randn energies (would
            # need a > 88 logit gap). Cross-partition sum+broadcast via a
            # ones-matmul on PE (cheaper than gpsimd on the critical tail).
            m1 = small.tile([EP, 1], f32)
            negm = small.tile([EP, 1], f32)
            expt = small.tile([EP, SCH], f32)
            sums = small.tile([EP, 1], f32)
            stl = small.tile([EP, 1], f32)
            ones = small.tile([EP, EP], f32)
            nc.vector.memset(ones[:], 1.0)
            nc.vector.reduce_max(m1[:], E_ps[:], axis=mybir.AxisListType.X)
            nc.gpsimd.partition_all_reduce(m1[:], m1[:], EP, ReduceOp.max)
            nc.scalar.mul(negm[:], m1[:], -1.0)
            nc.scalar.activation(
                expt[:, 0 : SCH - NTL],
                E_ps[:],
                mybir.ActivationFunctionType.Exp,
                bias=negm[:],
                accum_out=sums[:],
            )
            nc.scalar.activation(
                expt[:, SCH - NTL : SCH],
                Etl[:],
                mybir.ActivationFunctionType.Exp,
                bias=negm[:],
                accum_out=stl[:],
            )
            nc.vector.tensor_add(sums[:], sums[:], stl[:])
            total_ps = psum.tile([EP, 1], f32)
            nc.tensor.matmul(total_ps[:], ones[:], sums[:], start=True, stop=True)
            rs = small.tile([EP, 1], f32)
            nc.vector.reciprocal(rs[:], total_ps[:])
            nc.vector.tensor_scalar_mul(outt[0:EP, :], expt[:], rs[:])
            nc.gpsimd.trigger_dma(None)

    nc.compile()
    _fix_scatter_sem(nc)
    return nc


def _fix_scatter_sem(nc):
    """Point the scatter prep's descriptor-completion sem (on_update[0], our
    placeholder dsem) at the DMASW lane sem tile assigned to the prep. Tile's
    epilogue waits on that lane sem, but only the descriptor-baked sem fires
    at DMA completion — they must be the same sem, which the prepare_only API
    can't express (the lane is assigned during lowering)."""
    fn = nc.m.functions[0]
    insts = [i for bb in fn.blocks for i in bb.instructions]
    waited = {}
    updated = set()
    prep = None
    for i in insts:
        si = i.sync_info
        if not si:
            continue
        for u in si.on_update or []:
            updated.add(u.id)
        for w in si.on_wait or []:
            waited.setdefault(w.id, []).append(i.name)
        if type(i).__name__ == "InstDMAScatterAddAnt":
            prep = i
    assert prep is not None
    orphans = [sid for sid in waited if sid not in updated]
    assert len(orphans) == 1, (orphans, {k: waited[k] for k in orphans})
    si = prep.sync_info
    upd = list(si.on_update)
    first = upd[0]
    upd[0] = first.__replace__(id=orphans[0])
    prep.sync_info = si.__replace__(on_update=upd)


    # The scatter completes last (its trigger fires after the normalize), but
    # tile emitted its epilogue wait FIRST in the SP wait run — the ~8
    # trailing 50ns wait decodes then serialize after it. Rotate it to the
    # end of its run so the cheap waits decode while the scatter is in
    # flight.
    lane_id = orphans[0]
    for bb in fn.blocks:
        bl = list(bb.instructions)
        io = None
        for k, i in enumerate(bl):
            si2 = i.sync_info
            if (
                type(i).__name__ == "InstEventSemaphore"
                and si2
                and any(w.id == lane_id for w in (si2.on_wait or []))
            ):
                io = k
                break
        if io is None:
            continue
        i0 = io
        while (
            i0 > 0
            and type(bl[i0 - 1]).__name__ == "InstEventSemaphore"
            and bl[i0 - 1].engine == bl[io].engine
        ):
            i0 -= 1
        ie = io + 1
        while (
            ie < len(bl)
            and type(bl[ie]).__name__ == "InstEventSemaphore"
            and bl[ie].engine == bl[io].engine
        ):
            ie += 1
        run = bl[i0:ie]
        # Re-pack the exit wait conditions: the two LATE conds (the scatter
        # lane, which fires at transfer+0.9, and Pool_sequencer, which fires
        # at trigger+0.9) go together in the LAST EventSemaphore; all early
        # conds decode serially before the park instead of after the wake.
        conds = [w for i in run for w in (i.sync_info.on_wait or [])]
        late = [
            w
            for w in conds
            if w.id == lane_id or (w.ant_name or "").startswith("Pool_sequencer")
        ]
        early = [w for w in conds if w not in late]
        assert 1 <= len(late) <= 2 and len(late) + len(early) == len(conds)
        packs = []
        for k in range(len(run) - 1):
            take, early = early[:2], early[2:]
            packs.append(take)
        packs.append(early + late)
        assert all(1 <= len(p) <= 2 for p in packs), [len(p) for p in packs]
        for i, p in zip(run, packs):
            i.sync_info = i.sync_info.__replace__(on_wait=p)
        bb.instructions = bl
        break

    # The framework's const-AP memsets (no readers in this kernel) sit ahead
    # of the entry barrier and delay its release by ~0.4us. Move them past
    # the barrier so the first DMA issues immediately.
    bl0 = list(fn.blocks[0].instructions)
    movers = [
        i for i in bl0
        if type(i).__name__ == "InstMemset"
        and i.outs
        and getattr(
            getattr(getattr(i.outs[0], "bass_ap", None), "tensor", None),
            "name",
            "",
        ).startswith("const-")
    ]
    if movers:
        keep = [i for i in bl0 if i not in movers]
        fn.blocks[0].instructions = keep
        bl1 = list(fn.blocks[1].instructions)
        fn.blocks[1].instructions = movers + bl1

    # With the memsets gone, block 0's entry barrier synchronizes
    # nothing-to-nothing (exit barriers reuse the same gather/release sems
    # and the protocol is self-cleaning: both sems return to 0), so drop it
    # — engines branch straight into the body.
    bl0 = list(fn.blocks[0].instructions)
    fn.blocks[0].instructions = [
        i for i in bl0
        if type(i).__name__ not in ("InstDrain", "InstEventSemaphore")
    ]

    # The exit emits barrier / sem-reset / barrier. Round 2 only holds the
    # engines until Pool's reset retires, which the NEFF-completion protocol
    # (all queues drained before the next invocation) already guarantees —
    # drop it. Round 1 stays: it orders the reset after every in-flight sem
    # wait.
    lastbb = fn.blocks[-1]
    bl = list(lastbb.instructions)
    reset_idx = None
    for k, i in enumerate(bl):
        if type(i).__name__ == "InstDrain" and getattr(i, "is_reset_sema", False):
            reset_idx = k
    assert reset_idx is not None
    def _is_barrier(i):
        if type(i).__name__ not in ("InstDrain", "InstEventSemaphore"):
            return False
        si2 = i.sync_info
        if not si2:
            return False
        names = [w.ant_name or "" for w in (si2.on_wait or [])] + [
            u.ant_name or "" for u in (si2.on_update or [])
        ]
        return any(n.startswith("barrier_") for n in names)
    lastbb.instructions = bl[: reset_idx + 1] + [
        i for i in bl[reset_idx + 1 :] if not _is_barrier(i)
    ]

    # The exit's leading SP tick-drain (waits an engine tick satisfied tens
    # of microseconds earlier) decodes serially after the final parked wait;
    # the barrier's own SP Drain immediately follows and drains the same
    # pipeline. Drop the redundant one.
    bl = list(lastbb.instructions)
    for k, i in enumerate(bl):
        if (
            type(i).__name__ == "InstDrain"
            and str(i.engine) == "EngineType.SP"
            and i.sync_info
            and len(i.sync_info.on_wait or []) == 1
            and (i.sync_info.on_wait[0].ant_name or "").startswith("Pool_")
            and not (i.sync_info.on_update or [])
        ):
            del bl[k]
            lastbb.instructions = bl
            break


def _get_nc():
    global _cached_nc
    if _cached_nc is None:
        _cached_nc = _build()
    return _cached_nc


def _colperm():
    """colperm[j*128 + n] = (n%8)*128 + j*16 + n//8 — makes the RS output
    arrive p-major (v_lin[p*8 + j] = v[j*128 + p])."""
    hp = np.arange(H)
    j, n = hp // P, hp % P
    return (n % NCH) * P + j * (P // NCH) + n // NCH


def shard_inputs(inputs):
    """Per-core maps: core b gets batch b's enc (transposed + s-permuted to
    the PE tile layout), its o-slab of W (columns permuted for the RS->PE
    handoff), and the full hidden pre-transposed to the matmul lhsT layout."""
    hidden = np.asarray(inputs["hidden"], dtype=np.float32)
    enc = np.asarray(inputs["encoder_outputs"], dtype=np.float32)
    w = np.asarray(inputs["attn_w"], dtype=np.float32)
    # attn_b is a constant shift across s per batch -> cancels in softmax.
    w_perm = w[:, _colperm()]
    in_maps = []
    sa = np.zeros((P, 4), dtype=np.int16)
    for i in range(EP):
        sa[i % 16, i // 16] = i
    for b in range(NCORES):
        # encT[h, t*64 + p] = enc[p*64 + t, b, h]
        encT = np.ascontiguousarray(
            enc[:, b, :].reshape(EP, SCH, H).transpose(2, 1, 0).reshape(H, S)
        )
        in_maps.append(
            {
                "encT": encT,
                "wslab": np.ascontiguousarray(
                    np.concatenate(
                        [
                            w_perm[b * P : (b + 1) * P, :],
                            hidden[0, :, b * P : (b + 1) * P].T.astype(np.float32),
                            sa.view(np.float32),
                        ],
                        axis=1,
                    )
                ),
            }
        )
    return in_maps


def run(inputs, trace=False):
    """Shard, run SPMD on 8 cores, gather. Returns (output, BassKernelResults)."""
    nc = _get_nc()
    in_maps = shard_inputs(inputs)
    res = run_bass_kernel_spmd(
        nc, in_maps, core_ids=list(range(NCORES)), trace=trace
    )
    out = np.stack([res.results[b]["out"] for b in range(NCORES)], axis=0)
    return out[:, None, :].astype(np.float32), res


def kernel(hidden, encoder_outputs, attn_w, attn_b=None, **_unused):
    out, _ = run(
        {
            "hidden": hidden,
            "encoder_outputs": encoder_outputs,
            "attn_w": attn_w,
        }
    )
    return out


# revision 23
# speedup vs baseline: 1.0037x; 1.0012x over previous
"""Bass/Tile TRN2 kernel for nn_Attn: out = softmax_s(hidden . (W @ enc + b)).

Math: energies[b,s] = hidden[b] . (W enc[s,b] + bias) = (hidden[b] W) . enc[s,b] + const(b).
The const(b) term cancels in the softmax (and attn_b is zeros anyway), so per
batch element b:
    v_b = hidden[b] @ W            (tiny [1,H]x[H,H] GEMM)
    E[s] = enc[s, b, :] . v_b      (the 16.8 MB/core stream — the roofline)
    out[b, 0, :] = softmax_s(E)

Sharding: data-parallel over batch (core b owns batch b) for enc; the [H,H]
weight is sharded 8-way by o-rows instead of replicated (4 MiB -> 0.5 MiB per
core): each core computes partial v for ALL 8 batches from its o-slab on PE,
then one ReduceScatter ([8,H] -> [1,H]) hands core b exactly its own summed
v_b. The collective runs on the (modeled) collective cores, overlapped with
the enc stream, so the per-core DMA bill drops from 21 MiB to 17.3 MiB.

Energies on PE (not DVE): host pre-transposes enc to encT[h, s'] with
s' = t*64 + p <-> s = p*64 + t, streamed as 8 h-chunk buffers [128, 4096].
E[:, t] ([64, 64] layout) accumulates in PSUM over the 8 h-chunks via
[128K x 64M x 1N] matmuls (512 total), trailing the DMA stream by well under
a microsecond. Softmax: shift/exp/sums for the first 7 s-blocks' columns run
while the last enc block is in flight; only the last block's exps, the
normalize, and the output write trail the last byte. The cross-partition sum
uses a ones-matmul on PE (sum+broadcast in one op, cheaper than gpsimd on
the tail); the output leaves via a dma_scatter_add whose descriptors are
pre-generated at kernel start (scatter-add onto the pre-zeroed output region
== plain write), so only a trigger_dma (no HWDGE/DGE latency) sits in the
tail. The [64, 64] E layout exists because scatter rows must be 256-byte
aligned.

Host-side layout trickery (shard_inputs): W's columns are permuted so the
ReduceScatter output lands PE-ready: colperm[j*128 + n] = (n%8)*128 + j*16 +
n//8 makes v arrive as v_lin[p*8 + j] = v_b[j*128 + p], i.e. a plain
[128, 8] row-major load whose column j is exactly the h-chunk-j operand.

Post-compile fixups (_fix_scatter_sem): the prepare_only scatter's
descriptor-completion sem is rewritten to the DMASW lane sem tile assigned
(tile's epilogue waits that lane, but the prepare_only API can only bake a
user sem), and the epilogue's scatter wait is rotated to the end of the SP
wait run so the cheap lane waits decode while the scatter is in flight.
"""

import numpy as np

import concourse.bass as bass
import concourse.mybir as mybir
import concourse.tile as tile
from concourse import bacc
from concourse.bass_isa import ReduceOp
from concourse.bass_utils import run_bass_kernel_spmd

S, B, H = 4096, 8, 1024
P = 128
NCORES = 8
EP = 64               # E-tile partitions (s = p*64 + t, p in [0,64))
SCH = S // EP         # 64 energy columns per partition
NCH = H // P          # 8 h-chunks
SB = 512              # s-columns per enc DMA ([128, 512] = 256 KiB, 728 ns)
NSB = S // SB         # 8 s-blocks
TPB = SB // EP        # 8 s-tiles (PE M-tiles) per s-block

_cached_nc = None


def _build():
    nc = bacc.Bacc(
        "TRN2", target_bir_lowering=False, debug=False, num_devices=NCORES
    )
    f32 = mybir.dt.float32
    encT_d = nc.dram_tensor("encT", [H, S], f32, kind="ExternalInput")
    w_d = nc.dram_tensor("wslab", [P, H + B + 2], f32, kind="ExternalInput")
    out_d = nc.dram_tensor("out", [S], f32, kind="ExternalOutput")
    cc_in = nc.dram_tensor("cc_in", [B, H], f32, kind="Internal")
    cc_out = nc.dram_tensor("cc_out", [1, H], f32, kind="Internal")

    encT_r = encT_d.ap().rearrange("(c k) s -> c k s", c=NCH)  # [8, 128, 4096]
    out_zr = out_d.ap().rearrange("(p q) -> p q", p=32)        # [32, 128] zero-fill view (512B rows)
    out_sc = out_d.ap().rearrange("(r e) -> r e", e=EP)        # [64, 64] scatter rows
    vh_src = cc_out.ap().rearrange("one (p j) -> (one p) j", p=P)  # [128, 8]

    with tile.TileContext(nc) as tc:
        with (
            tc.tile_pool(name="enc", bufs=1) as encp,
            tc.tile_pool(name="small", bufs=1) as small,
            tc.tile_pool(name="psum", bufs=1, space=bass.MemorySpace.PSUM) as psum,
        ):
            # ---- v pipeline: partial vT for all 8 batches from this core's
            # o-slab, ReduceScatter, then a PE-ready [128, 8] reload.
            # one packed load: cols 0..1023 = W o-slab, 1024..1031 = hidT
            # (f32), 1032..1033 = scatter idxs (int16 x4, bitcast view)
            wpk = small.tile([P, H + B + 2], f32)
            nc.sync.dma_start(wpk[:], w_d.ap())
            wslab = wpk
            pk_off = H
            # ---- out path, prepared off the critical tail: pre-generate
            # scatter-add descriptors (scatter-add onto the zero-filled
            # output == plain write), fire via trigger_dma after the
            # normalize; tile's count=None path wires the data deps.
            zt = small.tile([32, P], f32)
            nc.vector.memset(zt[:], 0.0)
            outt = small.tile([P, SCH], f32)
            nc.vector.memset(outt[:], 0.0)
            dsem = nc.alloc_semaphore("dsem")
            nc.gpsimd.dma_scatter_add(
                out_sc,
                outt[:].rearrange("p (a f) -> p a f", a=1),
                wpk[:, pk_off + B : pk_off + B + 2].bitcast(mybir.dt.int16),
                num_idxs=EP,
                num_idxs_reg=EP,
                elem_size=EP,
                prepare_only=True,
                sem=dsem,
            )

            vps = psum.tile([B, H], f32)
            for j in range(NCH):
                nc.tensor.matmul(
                    vps[:, j * P : (j + 1) * P],
                    wpk[:, pk_off : pk_off + B],
                    wslab[:, j * P : (j + 1) * P],
                    start=True,
                    stop=True,
                )
            vsb = small.tile([B, H], f32)
            nc.vector.tensor_copy(vsb[:], vps[:])
            nc.gpsimd.dma_start(cc_in.ap(), vsb[:])
            nc.gpsimd.collective_compute(
                "ReduceScatter",
                mybir.AluOpType.add,
                [list(range(NCORES))],
                [cc_in.ap()],
                [cc_out.ap()],
            )
            vh = small.tile([P, B], f32)  # vh[p, j] = v_b[j*128 + p]
            nc.gpsimd.dma_start(vh[:], vh_src)


            # ---- enc stream: 8 resident h-chunk buffers, blocks of 512 s,
            # alternating two HWDGE queues
            qs = [nc.sync, nc.scalar]
            qi = 0
            encb = encp.tile([P, NCH, S], f32)
            for bs in range(NSB):
                sl = slice(bs * SB, (bs + 1) * SB)
                for c in range(NCH):
                    qs[qi % 2].dma_start(encb[:, c, sl], encT_r[c, :, sl])
                    qi += 1

            # zero-fill rides after the enc stream: its only consumer is the
            # scatter trigger at the very end, while the last enc byte gates
            # the energy/softmax tail.
            nc.sync.dma_start(out_zr, zt[:])

            # ---- energies: E[:, t] = sum_c encT_c[:, t-tile]^T @ vh[:, c].
            # The last s-block's 8 columns get their own PSUM tile so the
            # softmax prep's deps stop at the block boundary (col 55).
            NTL = TPB  # tail columns = one s-block
            E_ps = psum.tile([EP, SCH - NTL], f32)
            Etl = psum.tile([EP, NTL], f32)
            for t in range(SCH):
                dst = (
                    E_ps[:, t : t + 1]
                    if t < SCH - NTL
                    else Etl[:, t - (SCH - NTL) : t - (SCH - NTL) + 1]
                )
                for c in range(NCH):
                    nc.tensor.matmul(
                        dst,
                        encb[:, c, t * EP : (t + 1) * EP],
                        vh[:, c : c + 1],
                        start=(c == 0),
                        stop=(c == NCH - 1),
                    )

            # ---- softmax: shift/exp/sums for cols 0..27 run while the last
            # enc block is in flight; only the last block's 4 exps + the
            # normalize trail the last byte. The shift m~ = global max of
            # cols 0..27 is exact for softmax (shift-invariance); exp(E - m~)
            # of the last 4 cols cannot overflow for randn energies (would
            # need a > 88 logit gap). Cross-partition sum+broadcast via a
            # ones-matmul on PE (cheaper than gpsimd on the critical tail).
            m1 = small.tile([EP, 1], f32)
            negm = small.tile([EP, 1], f32)
            expt = small.tile([EP, SCH], f32)
            sums = small.tile([EP, 1], f32)
            stl = small.tile([EP, 1], f32)
            ones = small.tile([EP, EP], f32)
            nc.vector.memset(ones[:], 1.0)
            nc.vector.reduce_max(m1[:], E_ps[:], axis=mybir.AxisListType.X)
            nc.gpsimd.partition_all_reduce(m1[:], m1[:], EP, ReduceOp.max)
            nc.scalar.mul(negm[:], m1[:], -1.0)
            nc.scalar.activation(
                expt[:, 0 : SCH - NTL],
                E_ps[:],
                mybir.ActivationFunctionType.Exp,
                bias=negm[:],
                accum_out=sums[:],
            )
            nc.scalar.activation(
                expt[:, SCH - NTL : SCH],
                Etl[:],
                mybir.ActivationFunctionType.Exp,
                bias=negm[:],
                accum_out=stl[:],
            )
            nc.vector.tensor_add(sums[:], sums[:], stl[:])
            total_ps = psum.tile([EP, 1], f32)
            nc.tensor.matmul(total_ps[:], ones[:], sums[:], start=True, stop=True)
            rs = small.tile([EP, 1], f32)
            nc.vector.reciprocal(rs[:], total_ps[:])
            nc.vector.tensor_scalar_mul(outt[0:EP, :], expt[:], rs[:])
            nc.gpsimd.trigger_dma(None)

    nc.compile()
    _fix_scatter_sem(nc)
    return nc


def _fix_scatter_sem(nc):
    """Point the scatter prep's descriptor-completion sem (on_update[0], our
    placeholder dsem) at the DMASW lane sem tile assigned to the prep. Tile's
    epilogue waits on that lane sem, but only the descriptor-baked sem fires
    at DMA completion — they must be the same sem, which the prepare_only API
    can't express (the lane is assigned during lowering)."""
    fn = nc.m.functions[0]
    insts = [i for bb in fn.blocks for i in bb.instructions]
    waited = {}
    updated = set()
    prep = None
    for i in insts:
        si = i.sync_info
        if not si:
            continue
        for u in si.on_update or []:
            updated.add(u.id)
        for w in si.on_wait or []:
            waited.setdefault(w.id, []).append(i.name)
        if type(i).__name__ == "InstDMAScatterAddAnt":
            prep = i
    assert prep is not None
    orphans = [sid for sid in waited if sid not in updated]
    assert len(orphans) == 1, (orphans, {k: waited[k] for k in orphans})
    si = prep.sync_info
    upd = list(si.on_update)
    first = upd[0]
    upd[0] = first.__replace__(id=orphans[0])
    prep.sync_info = si.__replace__(on_update=upd)


    # The scatter completes last (its trigger fires after the normalize), but
    # tile emitted its epilogue wait FIRST in the SP wait run — the ~8
    # trailing 50ns wait decodes then serialize after it. Rotate it to the
    # end of its run so the cheap waits decode while the scatter is in
    # flight.
    lane_id = orphans[0]
    for bb in fn.blocks:
        bl = list(bb.instructions)
        io = None
        for k, i in enumerate(bl):
            si2 = i.sync_info
            if (
                type(i).__name__ == "InstEventSemaphore"
                and si2
                and any(w.id == lane_id for w in (si2.on_wait or []))
            ):
                io = k
                break
        if io is None:
            continue
        i0 = io
        while (
            i0 > 0
            and type(bl[i0 - 1]).__name__ == "InstEventSemaphore"
            and bl[i0 - 1].engine == bl[io].engine
        ):
            i0 -= 1
        ie = io + 1
        while (
            ie < len(bl)
            and type(bl[ie]).__name__ == "InstEventSemaphore"
            and bl[ie].engine == bl[io].engine
        ):
            ie += 1
        run = bl[i0:ie]
        # Re-pack the exit wait conditions: the two LATE conds (the scatter
        # lane, which fires at transfer+0.9, and Pool_sequencer, which fires
        # at trigger+0.9) go together in the LAST EventSemaphore; all early
        # conds decode serially before the park instead of after the wake.
        conds = [w for i in run for w in (i.sync_info.on_wait or [])]
        late = [
            w
            for w in conds
            if w.id == lane_id or (w.ant_name or "").startswith("Pool_sequencer")
        ]
        early = [w for w in conds if w not in late]
        assert 1 <= len(late) <= 2 and len(late) + len(early) == len(conds)
        packs = []
        for k in range(len(run) - 1):
            take, early = early[:2], early[2:]
            packs.append(take)
        packs.append(early + late)
        assert all(1 <= len(p) <= 2 for p in packs), [len(p) for p in packs]
        for i, p in zip(run, packs):
            i.sync_info = i.sync_info.__replace__(on_wait=p)
        bb.instructions = bl
        break

    # The framework's const-AP memsets (no readers in this kernel) sit ahead
    # of the entry barrier and delay its release by ~0.4us. Move them past
    # the barrier so the first DMA issues immediately.
    bl0 = list(fn.blocks[0].instructions)
    movers = [
        i for i in bl0
        if type(i).__name__ == "InstMemset"
        and i.outs
        and getattr(
            getattr(getattr(i.outs[0], "bass_ap", None), "tensor", None),
            "name",
            "",
        ).startswith("const-")
    ]
    if movers:
        keep = [i for i in bl0 if i not in movers]
        fn.blocks[0].instructions = keep
        bl1 = list(fn.blocks[1].instructions)
        fn.blocks[1].instructions = movers + bl1

    # With the memsets gone, block 0's entry barrier synchronizes
    # nothing-to-nothing (exit barriers reuse the same gather/release sems
    # and the protocol is self-cleaning: both sems return to 0), so drop it
    # — engines branch straight into the body.
    bl0 = list(fn.blocks[0].instructions)
    fn.blocks[0].instructions = [
        i for i in bl0
        if type(i).__name__ not in ("InstDrain", "InstEventSemaphore")
    ]

    # The exit emits barrier / sem-reset / barrier. Round 2 only holds the
    # engines until Pool's reset retires, which the NEFF-completion protocol
    # (all queues drained before the next invocation) already guarantees —
    # drop it. Round 1 stays: it orders the reset after every in-flight sem
    # wait.
    lastbb = fn.blocks[-1]
    bl = list(lastbb.instructions)
    reset_idx = None
    for k, i in enumerate(bl):
        if type(i).__name__ == "InstDrain" and getattr(i, "is_reset_sema", False):
            reset_idx = k
    assert reset_idx is not None
    def _is_barrier(i):
        if type(i).__name__ not in ("InstDrain", "InstEventSemaphore"):
            return False
        si2 = i.sync_info
        if not si2:
            return False
        names = [w.ant_name or "" for w in (si2.on_wait or [])] + [
            u.ant_name or "" for u in (si2.on_update or [])
        ]
        return any(n.startswith("barrier_") for n in names)
    lastbb.instructions = bl[: reset_idx + 1] + [
        i for i in bl[reset_idx + 1 :] if not _is_barrier(i)
    ]

    # The exit's leading SP tick-drain (waits an engine tick satisfied tens
    # of microseconds earlier) decodes serially after the final parked wait;
    # the barrier's own SP Drain immediately follows and drains the same
    # pipeline. Drop the redundant one.
    bl = list(lastbb.instructions)
    for k, i in enumerate(bl):
        if (
            type(i).__name__ == "InstDrain"
            and str(i.engine) == "EngineType.SP"
            and i.sync_info
            and len(i.sync_info.on_wait or []) == 1
            and (i.sync_info.on_wait[0].ant_name or "").startswith("Pool_")
            and not (i.sync_info.on_update or [])
        ):
            del bl[k]
            lastbb.instructions = bl
            break

    # The trigger's single ISA wait slot holds a trivially-satisfied Pool
    # tick while its gate EventSemaphore carries the late DVE tick (the
    # normalize), so the gate's decode serializes after that tick fires.
    # Swap them: gate decodes early, trigger parks directly on the DVE tick.
    insts2 = {i.name: i for bb2 in fn.blocks for i in bb2.instructions}
    trig = next(
        i for i in insts2.values() if type(i).__name__ == "InstTriggerDma"
    )
    gate = None
    for bb2 in fn.blocks:
        bl2 = list(bb2.instructions)
        for k2, i2 in enumerate(bl2):
            if i2.name == trig.name:
                for j2 in range(k2 - 1, -1, -1):
                    if str(bl2[j2].engine) != "EngineType.Pool":
                        continue
                    if type(bl2[j2]).__name__ == "InstEventSemaphore":
                        gate = bl2[j2]
                    break
    if gate is not None and gate.sync_info and trig.sync_info:
        gw = list(gate.sync_info.on_wait or [])
        tw = list(trig.sync_info.on_wait or [])
        gi = next(
            (
                k3
                for k3, w in enumerate(gw)
                if (w.ant_name or "").startswith("DVE")
            ),
            None,
        )
        if gi is not None and len(tw) == 1:
            gw[gi], tw[0] = tw[0], gw[gi]
            gate.sync_info = gate.sync_info.__replace__(on_wait=gw)
            trig.sync_info = trig.sync_info.__replace__(on_wait=tw)


def _get_nc():
    global _cached_nc
    if _cached_nc is None:
        _cached_nc = _build()
    return _cached_nc


def _colperm():
    """colperm[j*128 + n] = (n%8)*128 + j*16 + n//8 — makes the RS output
    arrive p-major (v_lin[p*8 + j] = v[j*128 + p])."""
    hp = np.arange(H)
    j, n = hp // P, hp % P
    return (n % NCH) * P + j * (P // NCH) + n // NCH


def shard_inputs(inputs):
    """Per-core maps: core b gets batch b's enc (transposed + s-permuted to
    the PE tile layout), its o-slab of W (columns permuted for the RS->PE
    handoff), and the full hidden pre-transposed to the matmul lhsT layout."""
    hidden = np.asarray(inputs["hidden"], dtype=np.float32)
    enc = np.asarray(inputs["encoder_outputs"], dtype=np.float32)
    w = np.asarray(inputs["attn_w"], dtype=np.float32)
    # attn_b is a constant shift across s per batch -> cancels in softmax.
    w_perm = w[:, _colperm()]
    in_maps = []
    sa = np.zeros((P, 4), dtype=np.int16)
    for i in range(EP):
        sa[i % 16, i // 16] = i
    for b in range(NCORES):
        # encT[h, t*64 + p] = enc[p*64 + t, b, h]
        encT = np.ascontiguousarray(
            enc[:, b, :].reshape(EP, SCH, H).transpose(2, 1, 0).reshape(H, S)
        )
        in_maps.append(
            {
                "encT": encT,
                "wslab": np.ascontiguousarray(
                    np.concatenate(
                        [
                            w_perm[b * P : (b + 1) * P, :],
                            hidden[0, :, b * P : (b + 1) * P].T.astype(np.float32),
                            sa.view(np.float32),
                        ],
                        axis=1,
                    )
                ),
            }
        )
    return in_maps


def run(inputs, trace=False):
    """Shard, run SPMD on 8 cores, gather. Returns (output, BassKernelResults)."""
    nc = _get_nc()
    in_maps = shard_inputs(inputs)
    res = run_bass_kernel_spmd(
        nc, in_maps, core_ids=list(range(NCORES)), trace=trace
    )
    out = np.stack([res.results[b]["out"] for b in range(NCORES)], axis=0)
    return out[:, None, :].astype(np.float32), res


def kernel(hidden, encoder_outputs, attn_w, attn_b=None, **_unused):
    out, _ = run(
        {
            "hidden": hidden,
            "encoder_outputs": encoder_outputs,
            "attn_w": attn_w,
        }
    )
    return out
